# revision 36
# baseline (speedup 1.0000x reference)
"""Multi-head self-attention (N=2, S=4096, D=1024, H=16) on 8 trn2 cores.

Sharding: data-parallel over batch (2) x tensor-parallel over head groups
(4 heads per core). Core c handles batch b=c//4, head group g=c%4
(heads 4g..4g+3, i.e. output columns 256g..256g+256). No cross-device
comms: heads are independent.

Per-core device kernel (v5):
  - Projections in fp16: qT/kT stored as head-PAIR planes [128, 2, S]
    (plane p rows 0:64 = head 2p dims, 64:128 = head 2p+1 — the natural
    projection layout), v in bf16 "vaug" (64 v cols + ones col per head
    for the free softmax denominator). Projection PSUM tiles borrow the
    ST pool slots (no separate proj pool).
  - ST per (pair, 512-query chunk, key chunk): TWO concurrent K=64
    row-tiled matmuls (tile_position (0,0)/(64,0) — distinct row groups
    overlap per-subarray), each M=128 keys x N=512 queries, into
    SEPARATE 1-bank PSUM tiles (Tile chains multiple readers of one
    tile serially, so the two exp engines need disjoint sources).
    3-deep ST double buffering hides the ST->exp->PV serial chain.
  - Softmax exp split across engines: ScalarE exact exp for head A,
    DVE Schraudolph fast-exp for head B (one tensor_scalar:
    round(A*score + B) -> int16 = bf16 bits with linearly-interpolated
    mantissa; +-4% weight noise that softmax normalization mostly
    cancels). Each engine owns its e-tile: fully parallel.
  - PV per head: vaug lhsT [128,128], rhs = that head's e tile, N=512,
    accumulated over 32 key chunks into one shared [128,1024] OT.
  - Epilogue per (pair, ic): one ScalarE copy OT->SBUF fp16, 8 XBAR
    DMA transposes (16x128-tile hardware transpose, off the PE),
    batched DVE reciprocal + broadcast multiply, strided DMA out.
  - PSUM (8 banks): stA 3 + stB 3 + OT 2 = 8.
"""

import numpy as np

import concourse.bacc as bacc
import concourse.tile as tile
import concourse.mybir as mybir
from concourse.bass_utils import run_bass_kernel_spmd
from concourse.masks import make_identity

F32 = mybir.dt.float32
BF16 = mybir.dt.bfloat16
FP16 = mybir.dt.float16
I16 = mybir.dt.int16
Exp = mybir.ActivationFunctionType.Exp
MULT = mybir.AluOpType.mult
ADD = mybir.AluOpType.add

N, S, D = 2, 4096, 1024
H = 16
HD = D // H                      # 64
N_CORES = 8
HPC = H // (N_CORES // N)        # heads per core = 4
MPC = HPC * HD                   # out columns per core = 256
SCALE = 1.0 / np.sqrt(HD)        # post-matmul softmax scale

IC = 512                         # queries per pair-unit
N_IC = S // IC                   # 8
N_JC = S // 128                  # 32 key chunks
N_SC = S // 512                  # 8 projection s-chunks
N_DT = D // 128                  # 8 contraction tiles
VW = HD + 1                      # vaug stride per head (64 v + 1 ones)
TP = 80                          # transpose rows: 65 used, 16-aligned

# Schraudolph fast-exp constants for bf16 bit pattern:
#   bits = round(score * SCALE * 128 * log2(e) + (127*128 - C))
EXP_A = float(SCALE * 128.0 * np.log2(np.e))
EXP_B = float(127.0 * 128.0 - 7.64)


def build_attention_kernel():
    nc = bacc.Bacc(
        "TRN2", target_bir_lowering=False, debug=False,
        enable_asserts=False, num_devices=N_CORES,
    )
    xT = nc.dram_tensor("xT", [D, S], FP16, kind="ExternalInput").ap()
    wqT = nc.dram_tensor("wqT", [D, MPC], FP16, kind="ExternalInput").ap()
    wkT = nc.dram_tensor("wkT", [D, MPC], FP16, kind="ExternalInput").ap()
    wvT = nc.dram_tensor("wvT", [D, MPC], FP16, kind="ExternalInput").ap()
    out = nc.dram_tensor("out", [S, MPC], F32, kind="ExternalOutput").ap()

    with tile.TileContext(nc) as tc:
        _emit(tc, xT, wqT, wkT, wvT, out)
    nc.compile()
    return nc


def _emit(tc, xT, wqT, wkT, wvT, out):
    nc = tc.nc
    with (
        tc.tile_pool(name="persist", bufs=1) as persist,
        # PSUM (8 banks): stA 2 + stB 2 + otA 1 + otB 1 + proj 2 = 8.
        # stA/stB bufs=2 keeps the two row-tiled STs of a unit becoming
        # ready together (bufs=3 let the scheduler split the pair; a
        # lone K=64 matmul half-idles the array and the HAM clock gate
        # throttles the whole PE to 1.2 GHz).
        tc.tile_pool(name="stp", bufs=2, space="PSUM") as stp,
        tc.tile_pool(name="otp", bufs=1, space="PSUM") as otp,
        tc.tile_pool(name="prp", bufs=2, space="PSUM") as prp,
        tc.tile_pool(name="xload", bufs=3) as xload,
        tc.tile_pool(name="esb", bufs=4) as esb,
        tc.tile_pool(name="episb", bufs=2) as episb,
        tc.tile_pool(name="osb", bufs=3) as osb,
    ):
        w_sb = {}
        for name, w in (("k", wkT), ("q", wqT), ("v", wvT)):
            t = persist.tile([128, N_DT, MPC], FP16, tag=f"w{name}")
            nc.sync.dma_start(
                out=t[:], in_=w[:].rearrange("(dt p) c -> p dt c", p=128))
            w_sb[name] = t
        # pair planes: plane p rows 0:64 = head 2p, rows 64:128 = head 2p+1
        qT_sb = persist.tile([128, 2, S], FP16, tag="qT")
        kTp = persist.tile([128, 2, S], FP16, tag="kTp")
        vaug = persist.tile([128, N_JC, HPC * VW + HD - 1], BF16, tag="vaug")
        ident = persist.tile([128, 128], F32, tag="ident")
        ones_src = persist.tile([128, HPC], F32, tag="ones")

        def init_vaug_ones():
            # emitted after the first projections so these DVE ops do
            # not outrank the projection copies in the scheduler
            nc.vector.memset(ones_src, 1.0)
            for jc in range(N_JC):      # vaug ones + zero pad columns
                nc.vector.tensor_copy(
                    vaug[:, jc, 0:HPC * VW].rearrange(
                        "p (h c) -> p h c", c=VW)[:, :, HD:HD + 1],
                    ones_src[:].rearrange("p (h c) -> p h c", c=1),
                )
                nc.vector.memset(vaug[:, jc, HPC * VW:], 0.0)

        # ---------- projection helpers ----------
        def load_x(sc):
            s0 = sc * 512
            x_t = xload.tile([128, N_DT, 512], FP16, tag="x")
            nc.sync.dma_start(
                out=x_t[:],
                in_=xT[:, s0:s0 + 512].rearrange("(dt p) c -> p dt c", p=128),
            )
            return x_t

        def proj_qk(sc, x_t, name):
            s0 = sc * 512
            dst = qT_sb if name == "q" else kTp
            for mt in range(2):
                ps = prp.tile([128, 512], F32, tag="pr")
                for dt in range(N_DT):
                    nc.tensor.matmul(
                        ps[:],
                        w_sb[name][:, dt, mt * 128:(mt + 1) * 128],
                        x_t[:, dt, :],
                        start=(dt == 0), stop=(dt == N_DT - 1),
                    )
                nc.any.tensor_copy(dst[:, mt, s0:s0 + 512], ps[:])

        def proj_v(sc, x_t):
            for st in range(4):
                ps = prp.tile([128, MPC], F32, tag="pr")
                for dt in range(N_DT):
                    nc.tensor.matmul(
                        ps[:],
                        x_t[:, dt, st * 128:(st + 1) * 128],
                        w_sb["v"][:, dt, :],
                        start=(dt == 0), stop=(dt == N_DT - 1),
                    )
                jc = sc * 4 + st
                nc.any.tensor_copy(
                    vaug[:, jc, 0:HPC * VW].rearrange(
                        "p (h c) -> p h c", c=VW)[:, :, 0:HD],
                    ps[:].rearrange("p (h d) -> p h d", d=HD),
                )

        # ---------- attention ----------
        def pair_unit(p, ic, jc, otA, otB):
            i0 = ic * IC
            j0 = jc * 128
            stA = stp.tile([128, 512], F32, tag="stA")
            stB = stp.tile([128, 512], F32, tag="stB")
            nc.tensor.matmul(
                stA[:],
                kTp[0:64, p, j0:j0 + 128],
                qT_sb[0:64, p, i0:i0 + 512],
                start=True, stop=True,
            )
            nc.tensor.matmul(
                stB[:],
                kTp[64:128, p, j0:j0 + 128],
                qT_sb[64:128, p, i0:i0 + 512],
                start=True, stop=True,
            )
            e_s = esb.tile([128, 512], BF16, tag="es")
            e_d = esb.tile([128, 512], BF16, tag="ed")
            nc.vector.tensor_scalar(
                out=e_d[:].bitcast(I16),
                in0=stB[:],
                scalar1=EXP_A, scalar2=EXP_B,
                op0=MULT, op1=ADD,
            )
            nc.scalar.activation(
                e_s[:], stA[:], Exp, bias=0.0, scale=SCALE)
            for hh, (ot_ps, e_t) in enumerate(((otA, e_s), (otB, e_d))):
                h = 2 * p + hh
                nc.tensor.matmul(
                    ot_ps[:],
                    vaug[:, jc, h * VW:h * VW + 128],
                    e_t[:],
                    start=(jc == 0), stop=(jc == N_JC - 1),
                )

        def epilogue(p, ic, otA, otB):
            i0 = ic * IC
            for hh, ot_ps in enumerate((otA, otB)):
                h = 2 * p + hh
                ot_sb = episb.tile([HD + 1, IC], F32, tag=f"eo{hh}")
                nc.scalar.copy(ot_sb[:], ot_ps[0:HD + 1, :])
                trAll = prp.tile([128, 4 * VW], F32, tag="pr")
                for bi in range(IC // 128):
                    nc.tensor.transpose(
                        trAll[:, bi * VW:(bi + 1) * VW],
                        ot_sb[:, bi * 128:(bi + 1) * 128],
                        ident[0:VW, 0:VW],
                    )
                trv = trAll[:].rearrange("p (b c) -> p b c", c=VW)
                rec = osb.tile([128, 4, 1], F32, tag=f"rec{hh}")
                nc.vector.reciprocal(rec[:], trv[:, :, HD:HD + 1])
                o_t = osb.tile([128, 4, HD], F32, tag=f"o{hh}")
                nc.vector.tensor_mul(
                    o_t[:], trv[:, :, 0:HD],
                    rec[:].broadcast_to([128, 4, HD]))
                nc.sync.dma_start(
                    out=out[i0:i0 + IC, h * HD:(h + 1) * HD].rearrange(
                        "(b q) d -> q b d", q=128),
                    in_=o_t[:],
                )

        # ---------- interleaved schedule ----------
        # single pass: k/q/v projections woven into (pair 0, ic 0)
        otA = otp.tile([128, IC], F32, tag="otA")
        otB = otp.tile([128, IC], F32, tag="otB")
        for sc in range(N_SC):
            x_t = load_x(sc)
            proj_qk(sc, x_t, "k")
            if sc == 0:
                proj_qk(sc, x_t, "q")
                init_vaug_ones()
                make_identity(nc, ident)
            proj_v(sc, x_t)
            for jc in range(sc * 4, sc * 4 + 4):
                pair_unit(0, 0, jc, otA, otB)
        epilogue(0, 0, otA, otB)
        # pair 1 / ic 0: remaining q projections woven in
        otA = otp.tile([128, IC], F32, tag="otA")
        otB = otp.tile([128, IC], F32, tag="otB")
        qsc = {0: 1, 4: 2, 8: 3, 12: 4, 16: 5, 20: 6, 24: 7}
        for jc in range(N_JC):
            if jc in qsc:
                x_t = load_x(qsc[jc])
                proj_qk(qsc[jc], x_t, "q")
            pair_unit(1, 0, jc, otA, otB)
        epilogue(1, 0, otA, otB)
        # the rest: pure attention
        for ic in range(1, N_IC):
            for p in range(2):
                otA = otp.tile([128, IC], F32, tag="otA")
                otB = otp.tile([128, IC], F32, tag="otB")
                for jc in range(N_JC):
                    pair_unit(p, ic, jc, otA, otB)
                epilogue(p, ic, otA, otB)


_NC_CACHE = None


def _get_nc():
    global _NC_CACHE
    if _NC_CACHE is None:
        _NC_CACHE = build_attention_kernel()
    return _NC_CACHE


def _build_in_maps(inputs):
    x = np.asarray(inputs["x"], dtype=np.float32)
    Wq = np.asarray(inputs["Wq"], dtype=np.float32)
    Wk = np.asarray(inputs["Wk"], dtype=np.float32)
    Wv = np.asarray(inputs["Wv"], dtype=np.float32)
    xTs = [np.ascontiguousarray(x[b].T).astype(np.float16)
           for b in range(N)]
    in_maps = []
    for c in range(N_CORES):
        b, g = divmod(c, N_CORES // N)
        rows = slice(g * MPC, (g + 1) * MPC)
        in_maps.append({
            "xT": xTs[b],
            "wqT": np.ascontiguousarray(Wq[rows].T).astype(np.float16),
            "wkT": np.ascontiguousarray(Wk[rows].T).astype(np.float16),
            "wvT": np.ascontiguousarray(Wv[rows].T).astype(np.float16),
        })
    return in_maps


def kernel(x, Wq, Wk, Wv):
    nc = _get_nc()
    in_maps = _build_in_maps({"x": x, "Wq": Wq, "Wk": Wk, "Wv": Wv})
    res = run_bass_kernel_spmd(nc, in_maps, core_ids=list(range(N_CORES)))

    full = np.empty((N, S, D), dtype=np.float32)
    for c in range(N_CORES):
        b, g = divmod(c, N_CORES // N)
        full[b, :, g * MPC:(g + 1) * MPC] = res.results[c]["out"]
    return full


if __name__ == "__main__":
    rng = np.random.default_rng(0)
    x = rng.standard_normal((N, S, D)).astype(np.float32)
    Wq = (rng.standard_normal((D, D)) / 32).astype(np.float32)
    Wk = (rng.standard_normal((D, D)) / 32).astype(np.float32)
    Wv = (rng.standard_normal((D, D)) / 32).astype(np.float32)
    got = kernel(x, Wq, Wk, Wv)
    print("kernel output:", got.shape, got.dtype)


# revision 38
# speedup vs baseline: 1.1907x; 1.1907x over previous
"""Multi-head self-attention (N=2, S=4096, D=1024, H=16) on 8 trn2 cores.

Sharding: data-parallel over batch (2) x tensor-parallel over head groups
(4 heads per core). Core c handles batch b=c//4, head group g=c%4
(heads 4g..4g+3, i.e. output columns 256g..256g+256). No cross-device
comms: heads are independent.

Per-core device kernel:
  - Projections in fp16, loaded/stored via one batched DMA per tensor:
    qT/kT as head-PAIR planes [128, 2, S] (plane p rows 0:64 = head 2p
    dims, rows 64:128 = head 2p+1 -- the natural projection layout),
    v in bf16 "vaug" (64 v cols + a ones col per head, so the PV
    matmul accumulates the softmax denominator for free).
  - ST per (pair, 512-query chunk, key chunk): TWO concurrent K=64
    row-tiled matmuls (tile_position (0,0)/(64,0) via base_partition;
    distinct row groups execute overlapped per-subarray), each M=128
    keys x N=512 queries. This halves ST PE time vs a zero-padded
    K=128 form. The two STs write SEPARATE 1-bank PSUM tiles because
    the Tile framework chains multiple readers of one tile serially --
    the two softmax engines must have disjoint sources to overlap.
  - Softmax exp split across engines, one writer per e-tile so they
    run fully parallel: ScalarE exact exp for head A; DVE Schraudolph
    fast-exp for head B (a single tensor_scalar:
    round(score*SCALE*128*log2e + 16248.36) -> int16, bit-identical
    to a bf16 with linearly-interpolated mantissa; +-4% weight noise
    that softmax normalization mostly cancels; measured end-to-end
    rel err 1.2e-2 vs the 2e-2 gate).
  - PV per head: vaug lhsT [128,128], rhs = that head's e-tile, N=512,
    accumulated over 32 key chunks into a 1-bank OT [128,512].
  - Epilogue per (pair, ic): ScalarE copy OT->SBUF, 4 PE fast
    LDW-transposes into one [128, 4*65] PSUM tile, one batched DVE
    reciprocal [128,4,1] + broadcast multiply, strided DMA out.
  - Single-pass schedule: k/q/v projections woven into the first two
    (pair, ic) groups so the softmax engines start ~15us in; x loads
    are single 1MB DMAs, triple-buffered.
  - PSUM (8 banks): stA 2 + stB 2 + otA 1 + otB 1 + proj/transpose 2.
  - Keep the ST pair adjacent in the PE stream: a lone K=64 matmul
    half-idles the array and the HAM clock gate throttles the PE to
    1.2 GHz (this killed two earlier variants; stA/stB bufs=2 with a
    shared free-event keeps the pair issuing back-to-back).
"""

import numpy as np

import concourse.bacc as bacc
import concourse.tile as tile
import concourse.mybir as mybir
from concourse.bass_utils import run_bass_kernel_spmd
from concourse.masks import make_identity

F32 = mybir.dt.float32
BF16 = mybir.dt.bfloat16
FP16 = mybir.dt.float16
I16 = mybir.dt.int16
Exp = mybir.ActivationFunctionType.Exp
MULT = mybir.AluOpType.mult
ADD = mybir.AluOpType.add

N, S, D = 2, 4096, 1024
H = 16
HD = D // H                      # 64
N_CORES = 8
HPC = H // (N_CORES // N)        # heads per core = 4
MPC = HPC * HD                   # out columns per core = 256
SCALE = 1.0 / np.sqrt(HD)        # post-matmul softmax scale

IC = 512                         # queries per pair-unit
N_IC = S // IC                   # 8
N_JC = S // 128                  # 32 key chunks
N_SC = S // 512                  # 8 projection s-chunks
N_DT = D // 128                  # 8 contraction tiles
VW = HD + 1                      # vaug stride per head (64 v + 1 ones)
TP = 80                          # transpose rows: 65 used, 16-aligned

# Schraudolph fast-exp constants for bf16 bit pattern:
#   bits = round(score * SCALE * 128 * log2(e) + (127*128 - C))
EXP_A = float(SCALE * 128.0 * np.log2(np.e))
EXP_B = float(127.0 * 128.0 - 7.64)


def build_attention_kernel():
    nc = bacc.Bacc(
        "TRN2", target_bir_lowering=False, debug=False,
        enable_asserts=False, num_devices=N_CORES,
    )
    xT = nc.dram_tensor("xT", [D, S], FP16, kind="ExternalInput").ap()
    wqT = nc.dram_tensor("wqT", [D, MPC], FP16, kind="ExternalInput").ap()
    wkT = nc.dram_tensor("wkT", [D, MPC], FP16, kind="ExternalInput").ap()
    wvT = nc.dram_tensor("wvT", [D, MPC], FP16, kind="ExternalInput").ap()
    out = nc.dram_tensor("out", [S, MPC], F32, kind="ExternalOutput").ap()

    with tile.TileContext(nc) as tc:
        _emit(tc, xT, wqT, wkT, wvT, out)
    nc.compile()
    return nc


def _emit(tc, xT, wqT, wkT, wvT, out):
    nc = tc.nc
    with (
        tc.tile_pool(name="persist", bufs=1) as persist,
        # PSUM (8 banks): stA 2 + stB 2 + otA 1 + otB 1 + proj 2 = 8.
        # stA/stB bufs=2 keeps the two row-tiled STs of a unit becoming
        # ready together (bufs=3 let the scheduler split the pair; a
        # lone K=64 matmul half-idles the array and the HAM clock gate
        # throttles the whole PE to 1.2 GHz).
        tc.tile_pool(name="stp", bufs=2, space="PSUM") as stp,
        tc.tile_pool(name="otp", bufs=1, space="PSUM") as otp,
        tc.tile_pool(name="prp", bufs=2, space="PSUM") as prp,
        tc.tile_pool(name="xload", bufs=3) as xload,
        tc.tile_pool(name="esb", bufs=4) as esb,
        tc.tile_pool(name="episb", bufs=2) as episb,
        tc.tile_pool(name="osb", bufs=3) as osb,
    ):
        w_sb = {}
        for name, w in (("k", wkT), ("q", wqT), ("v", wvT)):
            t = persist.tile([128, N_DT, MPC], FP16, tag=f"w{name}")
            nc.sync.dma_start(
                out=t[:], in_=w[:].rearrange("(dt p) c -> p dt c", p=128))
            w_sb[name] = t
        # pair planes: plane p rows 0:64 = head 2p, rows 64:128 = head 2p+1
        qT_sb = persist.tile([128, 2, S], FP16, tag="qT")
        kTp = persist.tile([128, 2, S], FP16, tag="kTp")
        vaug = persist.tile([128, N_JC, HPC * VW + HD - 1], BF16, tag="vaug")
        ident = persist.tile([128, 128], F32, tag="ident")
        make_identity(nc, ident)

        ones_src = persist.tile([128, HPC], F32, tag="ones")
        nc.vector.memset(ones_src, 1.0)
        for jc in range(N_JC):          # vaug ones + zero pad columns
            nc.vector.tensor_copy(
                vaug[:, jc, 0:HPC * VW].rearrange(
                    "p (h c) -> p h c", c=VW)[:, :, HD:HD + 1],
                ones_src[:].rearrange("p (h c) -> p h c", c=1),
            )
            nc.vector.memset(vaug[:, jc, HPC * VW:], 0.0)

        # ---------- projection helpers ----------
        def load_x(sc):
            s0 = sc * 512
            x_t = xload.tile([128, N_DT, 512], FP16, tag="x")
            nc.sync.dma_start(
                out=x_t[:],
                in_=xT[:, s0:s0 + 512].rearrange("(dt p) c -> p dt c", p=128),
            )
            return x_t

        def proj_qk(sc, x_t, name):
            s0 = sc * 512
            dst = qT_sb if name == "q" else kTp
            for mt in range(2):
                ps = prp.tile([128, 512], F32, tag="pr")
                for dt in range(N_DT):
                    nc.tensor.matmul(
                        ps[:],
                        w_sb[name][:, dt, mt * 128:(mt + 1) * 128],
                        x_t[:, dt, :],
                        start=(dt == 0), stop=(dt == N_DT - 1),
                    )
                nc.any.tensor_copy(dst[:, mt, s0:s0 + 512], ps[:])

        def proj_v(sc, x_t):
            for st in range(4):
                ps = prp.tile([128, MPC], F32, tag="pr")
                for dt in range(N_DT):
                    nc.tensor.matmul(
                        ps[:],
                        x_t[:, dt, st * 128:(st + 1) * 128],
                        w_sb["v"][:, dt, :],
                        start=(dt == 0), stop=(dt == N_DT - 1),
                    )
                jc = sc * 4 + st
                nc.any.tensor_copy(
                    vaug[:, jc, 0:HPC * VW].rearrange(
                        "p (h c) -> p h c", c=VW)[:, :, 0:HD],
                    ps[:].rearrange("p (h d) -> p h d", d=HD),
                )

        # ---------- attention ----------
        def pair_unit(p, ic, jc, otA, otB):
            i0 = ic * IC
            j0 = jc * 128
            stA = stp.tile([128, 512], F32, tag="stA")
            stB = stp.tile([128, 512], F32, tag="stB")
            nc.tensor.matmul(
                stA[:],
                kTp[0:64, p, j0:j0 + 128],
                qT_sb[0:64, p, i0:i0 + 512],
                start=True, stop=True,
            )
            nc.tensor.matmul(
                stB[:],
                kTp[64:128, p, j0:j0 + 128],
                qT_sb[64:128, p, i0:i0 + 512],
                start=True, stop=True,
            )
            e_s = esb.tile([128, 512], BF16, tag="es")
            e_d = esb.tile([128, 512], BF16, tag="ed")
            nc.vector.tensor_scalar(
                out=e_d[:].bitcast(I16),
                in0=stB[:],
                scalar1=EXP_A, scalar2=EXP_B,
                op0=MULT, op1=ADD,
            )
            nc.scalar.activation(
                e_s[:], stA[:], Exp, bias=0.0, scale=SCALE)
            for hh, (ot_ps, e_t) in enumerate(((otA, e_s), (otB, e_d))):
                h = 2 * p + hh
                nc.tensor.matmul(
                    ot_ps[:],
                    vaug[:, jc, h * VW:h * VW + 128],
                    e_t[:],
                    start=(jc == 0), stop=(jc == N_JC - 1),
                )

        def epilogue(p, ic, otA, otB):
            i0 = ic * IC
            for hh, ot_ps in enumerate((otA, otB)):
                h = 2 * p + hh
                ot_sb = episb.tile([HD + 1, IC], F32, tag=f"eo{hh}")
                nc.scalar.copy(ot_sb[:], ot_ps[0:HD + 1, :])
                trAll = prp.tile([128, 4 * VW], F32, tag="pr")
                for bi in range(IC // 128):
                    nc.tensor.transpose(
                        trAll[:, bi * VW:(bi + 1) * VW],
                        ot_sb[:, bi * 128:(bi + 1) * 128],
                        ident[0:VW, 0:VW],
                    )
                trv = trAll[:].rearrange("p (b c) -> p b c", c=VW)
                rec = osb.tile([128, 4, 1], F32, tag=f"rec{hh}")
                nc.vector.reciprocal(rec[:], trv[:, :, HD:HD + 1])
                o_t = osb.tile([128, 4, HD], F32, tag=f"o{hh}")
                nc.vector.tensor_mul(
                    o_t[:], trv[:, :, 0:HD],
                    rec[:].broadcast_to([128, 4, HD]))
                nc.sync.dma_start(
                    out=out[i0:i0 + IC, h * HD:(h + 1) * HD].rearrange(
                        "(b q) d -> q b d", q=128),
                    in_=o_t[:],
                )

        # ---------- interleaved schedule ----------
        # single pass: k/q/v projections woven into (pair 0, ic 0)
        otA = otp.tile([128, IC], F32, tag="otA")
        otB = otp.tile([128, IC], F32, tag="otB")
        for sc in range(N_SC):
            x_t = load_x(sc)
            proj_qk(sc, x_t, "k")
            if sc == 0:
                proj_qk(sc, x_t, "q")
            proj_v(sc, x_t)
            for jc in range(sc * 4, sc * 4 + 4):
                pair_unit(0, 0, jc, otA, otB)
        epilogue(0, 0, otA, otB)
        # pair 1 / ic 0: remaining q projections woven in
        otA = otp.tile([128, IC], F32, tag="otA")
        otB = otp.tile([128, IC], F32, tag="otB")
        qsc = {0: 1, 4: 2, 8: 3, 12: 4, 16: 5, 20: 6, 24: 7}
        for jc in range(N_JC):
            if jc in qsc:
                x_t = load_x(qsc[jc])
                proj_qk(qsc[jc], x_t, "q")
            pair_unit(1, 0, jc, otA, otB)
        epilogue(1, 0, otA, otB)
        # the rest: pure attention
        for ic in range(1, N_IC):
            for p in range(2):
                otA = otp.tile([128, IC], F32, tag="otA")
                otB = otp.tile([128, IC], F32, tag="otB")
                for jc in range(N_JC):
                    pair_unit(p, ic, jc, otA, otB)
                epilogue(p, ic, otA, otB)


_NC_CACHE = None


def _get_nc():
    global _NC_CACHE
    if _NC_CACHE is None:
        _NC_CACHE = build_attention_kernel()
    return _NC_CACHE


def _build_in_maps(inputs):
    x = np.asarray(inputs["x"], dtype=np.float32)
    Wq = np.asarray(inputs["Wq"], dtype=np.float32)
    Wk = np.asarray(inputs["Wk"], dtype=np.float32)
    Wv = np.asarray(inputs["Wv"], dtype=np.float32)
    xTs = [np.ascontiguousarray(x[b].T).astype(np.float16)
           for b in range(N)]
    in_maps = []
    for c in range(N_CORES):
        b, g = divmod(c, N_CORES // N)
        rows = slice(g * MPC, (g + 1) * MPC)
        in_maps.append({
            "xT": xTs[b],
            "wqT": np.ascontiguousarray(Wq[rows].T).astype(np.float16),
            "wkT": np.ascontiguousarray(Wk[rows].T).astype(np.float16),
            "wvT": np.ascontiguousarray(Wv[rows].T).astype(np.float16),
        })
    return in_maps


def kernel(x, Wq, Wk, Wv):
    nc = _get_nc()
    in_maps = _build_in_maps({"x": x, "Wq": Wq, "Wk": Wk, "Wv": Wv})
    res = run_bass_kernel_spmd(nc, in_maps, core_ids=list(range(N_CORES)))

    full = np.empty((N, S, D), dtype=np.float32)
    for c in range(N_CORES):
        b, g = divmod(c, N_CORES // N)
        full[b, :, g * MPC:(g + 1) * MPC] = res.results[c]["out"]
    return full


if __name__ == "__main__":
    rng = np.random.default_rng(0)
    x = rng.standard_normal((N, S, D)).astype(np.float32)
    Wq = (rng.standard_normal((D, D)) / 32).astype(np.float32)
    Wk = (rng.standard_normal((D, D)) / 32).astype(np.float32)
    Wv = (rng.standard_normal((D, D)) / 32).astype(np.float32)
    got = kernel(x, Wq, Wk, Wv)
    print("kernel output:", got.shape, got.dtype)
